# revision 20
# baseline (speedup 1.0000x reference)
# Multi-head attention (RoPE, causal) Trainium2 Bass kernel, v2.
# B=2, S=2048, D=1024, 16 heads, hd=64, fp32 I/O.
#
# Sharding: 32 (batch, head) units over 8 cores -> each core gets one batch
# and 4 heads. Each core computes its 4 heads' attention output and the
# partial out-projection (sum over its heads); the host sums the 4 partials
# per batch and adds the bias constant.
#
# v2 vs v1: Q^T/K^T are produced directly transposed by the QKV projection
# (W stationary, X^T streaming) so no PE transposes are needed; RoPE runs in
# the transposed [hd, s] layout using a host-side W-row permutation that
# makes the rotation partner swap a single DVE stream_shuffle (swap the
# 16-halves of each 32-partition block); score matmuls are K=64 row-packed
# pairs (partitions 0:64 / 64:128) that execute concurrently on the PE;
# diagonal score blocks are N-trimmed; A/B phases are interleaved per
# 512-chunk to keep the PE dense.
#
# Self-contained: all shapes/sharding hardcoded; no sibling imports.

import numpy as np

import concourse.bass as bass  # noqa: F401
import concourse.mybir as mybir
import concourse.tile as tile
from concourse import bacc, bass_utils

F32 = mybir.dt.float32
BF16 = mybir.dt.bfloat16
EXP = mybir.ActivationFunctionType.Exp
ADD = mybir.AluOpType.add
MULT = mybir.AluOpType.mult

B = 2
S = 2048
D = 1024
NHEADS = 16
HD = 64
HPC = 4  # heads per core
NCORES = 8
NPAIR = 2  # head pairs per core
P = 128
CH = 512  # q chunk
NCH = S // CH  # 4
NT = S // P  # 16
KT = D // P  # 8
THETA = 10000.0

# swap the 16-halves of each 32-partition block (RoPE partner swap)
SWAP_MASK = list(range(16, 32)) + list(range(0, 16))

# module-level knobs for test harness
TRACE = False
LAST_RESULTS = None

_PROGRAM_CACHE = {}


def build_program():
    nc = bacc.Bacc(
        "TRN2", target_bir_lowering=False, debug=False, enable_asserts=False
    )

    # ---- DRAM I/O ----
    xt_d = nc.dram_tensor("xt", [P, KT * S], BF16, kind="ExternalInput").ap()
    wq_d = nc.dram_tensor("wq", [P, NPAIR * KT * P], BF16, kind="ExternalInput").ap()
    wk_d = nc.dram_tensor("wk", [P, NPAIR * KT * P], BF16, kind="ExternalInput").ap()
    wv_d = nc.dram_tensor("wv", [P, KT * 256], BF16, kind="ExternalInput").ap()
    bqk_d = nc.dram_tensor("bqk", [P, 4], F32, kind="ExternalInput").ap()
    ropec_d = nc.dram_tensor("ropec", [P, S], BF16, kind="ExternalInput").ap()
    ropes_d = nc.dram_tensor("ropes", [P, S], BF16, kind="ExternalInput").ap()
    trimask_d = nc.dram_tensor("trimask", [P, 2 * P], BF16, kind="ExternalInput").ap()
    wo_d = nc.dram_tensor("wo", [P, NPAIR * D], BF16, kind="ExternalInput").ap()
    out_d = nc.dram_tensor("outp", [S, D], F32, kind="ExternalOutput").ap()

    from contextlib import ExitStack

    with tile.TileContext(nc) as tc, ExitStack() as ctx:
        const = ctx.enter_context(tc.tile_pool(name="const", bufs=1))

        # weights for Q^T/K^T: lhsT tiles [128 (D-chunk k), 128 (pack rows)]
        wq_sb = const.tile([P, NPAIR * KT * P], BF16)
        wk_sb = const.tile([P, NPAIR * KT * P], BF16)
        wv_sb = const.tile([P, KT * 256], BF16)
        bqk_sb = const.tile([P, 4], F32)
        ropec_sb = const.tile([P, S], BF16)
        ropes_sb = const.tile([P, S], BF16)
        trimask_sb = const.tile([P, 2 * P], BF16)
        wo_sb = const.tile([P, NPAIR * D], BF16)
        # x^T, entire input: [128 (D-chunk k), k*S + t]
        xt_sb = const.tile([P, KT * S], BF16)

        # const loads: split across the two HWDGE rings; first-needed first.
        # sync ring: wq pack0, x chunk 0 k-slices, wq pack1, rest of x
        nc.sync.dma_start(wq_sb[:, 0 : KT * P], wq_d[:, 0 : KT * P])
        for k in range(KT):
            sl = slice(k * S, k * S + CH)
            nc.sync.dma_start(xt_sb[:, sl], xt_d[:, sl])
        nc.sync.dma_start(wq_sb[:, KT * P :], wq_d[:, KT * P :])
        for c in range(1, NCH):
            for k in range(KT):
                sl = slice(k * S + c * CH, k * S + (c + 1) * CH)
                nc.sync.dma_start(xt_sb[:, sl], xt_d[:, sl])
        # scalar/ACT ring: everything else
        nc.scalar.dma_start(wk_sb[:], wk_d[:])
        nc.scalar.dma_start(bqk_sb[:], bqk_d[:])
        nc.scalar.dma_start(ropec_sb[:], ropec_d[:])
        nc.scalar.dma_start(ropes_sb[:], ropes_d[:])
        nc.scalar.dma_start(wv_sb[:], wv_d[:])
        nc.scalar.dma_start(trimask_sb[:], trimask_d[:])
        nc.scalar.dma_start(wo_sb[:], wo_d[:])

        # persistent activations
        # Q^T / K^T packs: [128 (2 heads x 64 permuted hd), s] per pack
        qt = const.tile([P, NPAIR * S], BF16)
        kt_sb = const.tile([P, NPAIR * S], BF16)
        # vone: per s-tile [128, 4*65]; per head 64 V cols + ones col
        vone = const.tile([P, NT * (HPC * 65)], BF16)
        # O^T packs (normalized in place): [128 (2 heads*hd), s] per pair
        otn2 = const.tile([P, NPAIR * S], BF16)

        vone_v = vone.rearrange("p (t h c) -> p t h c", t=NT, h=HPC)
        nc.gpsimd.memset(vone_v[:, :, :, 64], 1.0)

        # half-selector columns for the rank-1 denominator broadcast
        halfsel = const.tile([1, 256], BF16)
        nc.gpsimd.memset(halfsel[:, 0:64], 1.0)
        nc.gpsimd.memset(halfsel[:, 64:192], 0.0)
        nc.gpsimd.memset(halfsel[:, 192:256], 1.0)

        # softmax denominator collection: per (pair, J): [headA 512 | headB 512]
        dall = const.tile([1, NPAIR * NCH * 1024], F32)
        dallinv = const.tile([1, NPAIR * NCH * 1024], BF16)

        bpsum = ctx.enter_context(tc.tile_pool(name="bpsum", bufs=2, space="PSUM"))
        opsum = ctx.enter_context(tc.tile_pool(name="opsum", bufs=1, space="PSUM"))
        awork = ctx.enter_context(tc.tile_pool(name="awork", bufs=3))
        bwork = ctx.enter_context(tc.tile_pool(name="bwork", bufs=4))
        fxwork = ctx.enter_context(tc.tile_pool(name="fxwork", bufs=2))
        cwork = ctx.enter_context(tc.tile_pool(name="cwork", bufs=3))

        # ---------------- Phase A ----------------
        def emit_qkT(c, pk, w_sb, bias_col, dst):
            """One 512-col chunk of a Q^T or K^T pack, with bias + RoPE."""
            ps = bpsum.tile([P, CH], F32, name="psq", tag="proj")
            for k in range(KT):
                nc.tensor.matmul(
                    ps[:],
                    lhsT=w_sb[:, (pk * KT + k) * P : (pk * KT + k + 1) * P],
                    rhs=xt_sb[:, k * S + c * CH : k * S + (c + 1) * CH],
                    start=(k == 0),
                    stop=(k == KT - 1),
                )
            # stage = ps + bias (per-partition)
            stage = awork.tile([P, CH], BF16, name="stage")
            nc.vector.tensor_scalar_add(stage[:], ps[:], bqk_sb[:, bias_col : bias_col + 1])
            # t1 = stage * S'' ; sw = shuffle(t1) ; rot = stage*C + sw
            t1 = awork.tile([P, CH], BF16, name="t1")
            nc.vector.tensor_mul(t1[:], stage[:], ropes_sb[:, c * CH : (c + 1) * CH])
            sw = awork.tile([P, CH], BF16, name="sw")
            nc.vector.stream_shuffle(sw[:], t1[:], SWAP_MASK)
            rot1 = awork.tile([P, CH], BF16, name="rot1")
            nc.vector.tensor_mul(rot1[:], stage[:], ropec_sb[:, c * CH : (c + 1) * CH])
            nc.vector.tensor_add(
                dst[:, pk * S + c * CH : pk * S + (c + 1) * CH], rot1[:], sw[:]
            )

        def emit_V(st):
            psv = bpsum.tile([P, 256], F32, name="psv", tag="proj")
            for k in range(KT):
                nc.tensor.matmul(
                    psv[:],
                    lhsT=xt_sb[:, k * S + st * P : k * S + (st + 1) * P],
                    rhs=wv_sb[:, k * 256 : (k + 1) * 256],
                    start=(k == 0),
                    stop=(k == KT - 1),
                )
            nc.scalar.copy(
                vone_v[:, st, :, 0:64],
                psv.rearrange("p (h c) -> p h c", h=HPC),
            )

        # ---------------- Phase B: attention ----------------
        def emit_BJ(p, j, a_units=()):
            """One (pair, q-chunk) attention pass. a_units: deferred Phase-A
            sub-unit closures sprinkled between m-iterations so the PE's
            A-work fills the slack under ACT-bound exp."""
            units = list(a_units)
            n_done = 0
            q_pack = qt[:, p * S : (p + 1) * S]
            k_pack = kt_sb[:, p * S : (p + 1) * S]
            ot2 = opsum.tile([P, 1024], F32, name="ot2", tag="acc")
            mlast = 4 * j + 3
            n_m = 4 * j + 4
            for m in range(n_m):
                off = m * P - j * CH if m >= 4 * j else 0
                sc2 = bpsum.tile([P, 1024], F32, name="sc2", tag="sc")
                nc.tensor.matmul(
                    sc2[:, off:CH],
                    lhsT=k_pack[0:64, m * P : (m + 1) * P],
                    rhs=q_pack[0:64, j * CH + off : (j + 1) * CH],
                )
                nc.tensor.matmul(
                    sc2[:, CH + off : 1024],
                    lhsT=k_pack[64:128, m * P : (m + 1) * P],
                    rhs=q_pack[64:128, j * CH + off : (j + 1) * CH],
                )
                at2 = bwork.tile([P, 1024], BF16, name="at2")
                if off > 0:
                    sc_v = sc2.rearrange("p (h q) -> p h q", h=2)
                    at_v = at2.rearrange("p (h q) -> p h q", h=2)
                    nc.scalar.activation(
                        at_v[:, :, off:CH], sc_v[:, :, off:CH], EXP, scale=0.125
                    )
                else:
                    nc.scalar.activation(at2[:], sc2[:], EXP, scale=0.125)
                if m >= 4 * j:
                    at_m = at2.rearrange("p (h q) -> p h q", h=2)[
                        :, :, off : off + P
                    ]
                    nc.vector.tensor_mul(
                        at_m,
                        at_m,
                        trimask_sb.rearrange("p (h q) -> p h q", h=2),
                    )
                nc.tensor.matmul(
                    ot2[0:65, off:CH],
                    lhsT=vone_v[:, m, 2 * p, :],
                    rhs=at2[:, off:CH],
                    start=(m == 0),
                    stop=(m == mlast),
                )
                nc.tensor.matmul(
                    ot2[0:65, CH + off : 1024],
                    lhsT=vone_v[:, m, 2 * p + 1, :],
                    rhs=at2[:, CH + off : 1024],
                    start=(m == 0),
                    stop=(m == mlast),
                )
                while n_done < len(units) * (m + 1) // n_m:
                    units[n_done]()
                    n_done += 1
            # ---- evict OT halves + denominators (per J) ----
            ctx_hp = tc.high_priority(offset=600)
            ctx_hp.__enter__()
            nc.scalar.copy(
                otn2[0:64, p * S + j * CH : p * S + (j + 1) * CH], ot2[0:64, 0:CH]
            )
            stgB = fxwork.tile([64, CH], BF16, name="stgB")
            nc.vector.tensor_copy(stgB[:], ot2[0:64, CH:1024])
            nc.sync.dma_start(
                otn2[64:128, p * S + j * CH : p * S + (j + 1) * CH], stgB[:]
            )
            dslot = (p * NCH + j) * 1024
            nc.vector.tensor_copy(
                dall[0:1, dslot : dslot + CH], ot2[64:65, 0:CH]
            )
            nc.scalar.copy(
                dall[0:1, dslot + CH : dslot + 1024], ot2[64:65, CH:1024]
            )
            # reciprocal in partition-parallel layout
            dPj = fxwork.tile([P, 8], F32, name="dPj")
            nc.sync.dma_start(
                dPj[:],
                dall[0:1, dslot : dslot + 1024].rearrange("o (a b) -> o a b", a=P),
            )
            dPq = fxwork.tile([P, 8], F32, name="dPq")
            nc.vector.reciprocal(dPq[:], dPj[:])
            dPc = fxwork.tile([P, 8], BF16, name="dPc")
            with nc.allow_low_precision("softmax denominators"):
                nc.vector.tensor_copy(dPc[:], dPq[:])
            nc.sync.dma_start(
                dallinv[0:1, dslot : dslot + 1024].rearrange("o (a b) -> o a b", a=P),
                dPc[:],
            )
            ctx_hp.__exit__(None, None, None)

        def emit_final(p, j):
            dslot = (p * NCH + j) * 1024
            dvb = bpsum.tile([P, CH], F32, name="dvb", tag="proj")
            nc.tensor.matmul(
                dvb[:],
                lhsT=halfsel[0:1, 0:128],
                rhs=dallinv[0:1, dslot : dslot + CH],
                start=True,
                stop=False,
            )
            nc.tensor.matmul(
                dvb[:],
                lhsT=halfsel[0:1, 128:256],
                rhs=dallinv[0:1, dslot + CH : dslot + 1024],
                start=False,
                stop=True,
            )
            nc.vector.tensor_mul(
                otn2[:, p * S + j * CH : p * S + (j + 1) * CH],
                otn2[:, p * S + j * CH : p * S + (j + 1) * CH],
                dvb[:],
            )

        def emit_C(g):
            # out projection for q-tiles 4g..4g+3
            for qt_i in range(4 * g, 4 * g + 4):
                outsb = cwork.tile([P, D], F32, name="outsb")
                for dc in range(2):
                    pr = bpsum.tile([P, CH], F32, name="pr", tag="proj")
                    for p in range(NPAIR):
                        nc.tensor.matmul(
                            pr[:],
                            lhsT=otn2[:, p * S + qt_i * P : p * S + (qt_i + 1) * P],
                            rhs=wo_sb[:, p * D + dc * CH : p * D + (dc + 1) * CH],
                            start=(p == 0),
                            stop=(p == NPAIR - 1),
                        )
                    if dc == 0:
                        nc.vector.tensor_copy(outsb[:, 0:CH], pr[:])
                    else:
                        nc.scalar.copy(outsb[:, CH:D], pr[:])
                nc.gpsimd.dma_start(out_d[qt_i * P : (qt_i + 1) * P, :], outsb[:])

        # ---------------- schedule ----------------
        # A(0) pack0+V prefix; the rest of Phase A is deferred as sub-unit
        # closures sprinkled into the B m-loops (PE fills the slack under
        # ACT-bound exp). Chunk c's pack0+V units finish inside B(*, c-1)
        # before B(0, c); pack1 units before B(1, c). Finals pipeline one
        # (p, j) behind; out-projection per chunk after pair-1 finals.
        def unit_qkT(c, pk, w_sb, bias_col, dst):
            return lambda: emit_qkT(c, pk, w_sb, bias_col, dst)

        def unit_V(st):
            return lambda: emit_V(st)

        def units_p0(c):
            return [
                unit_qkT(c, 0, wq_sb, 0, qt),
                unit_qkT(c, 0, wk_sb, 2, kt_sb),
            ] + [unit_V(st) for st in range(4 * c, 4 * c + 4)]

        def units_p1(c):
            return [
                unit_qkT(c, 1, wq_sb, 1, qt),
                unit_qkT(c, 1, wk_sb, 3, kt_sb),
            ]

        emit_qkT(0, 0, wq_sb, 0, qt)
        emit_qkT(0, 0, wk_sb, 2, kt_sb)
        for st in range(4):
            emit_V(st)
        pending = None

        def after_B(p, j):
            nonlocal pending
            if pending is not None:
                emit_final(*pending)
                if pending[0] == 1:
                    emit_C(pending[1])
            pending = (p, j)

        emit_qkT(0, 1, wq_sb, 1, qt)
        emit_qkT(0, 1, wk_sb, 3, kt_sb)
        for j in range(NCH - 1):
            emit_BJ(0, j)
            after_B(0, j)
            for u in units_p0(j + 1):
                u()
            emit_BJ(1, j)
            after_B(1, j)
            for u in units_p1(j + 1):
                u()
        # All out-projection chunks are deferred into the j=3 region: their
        # matmuls are always-ready fillers for the PE stalls of the pure-B
        # last chunk (no Phase-A work remains there) and cover the last
        # fixup chains' latency, keeping the PE warm into C(3).
        emit_BJ(0, NCH - 1)
        after_B(0, NCH - 1)
        emit_BJ(1, NCH - 1)
        after_B(1, NCH - 1)
        emit_final(*pending)
        emit_C(pending[1])

    nc.compile()
    return nc


def get_program():
    if "v2" not in _PROGRAM_CACHE:
        _PROGRAM_CACHE["v2"] = build_program()
    return _PROGRAM_CACHE["v2"]


def _bf16(a):
    import ml_dtypes

    return np.ascontiguousarray(a).astype(ml_dtypes.bfloat16)


def _rope_perm64():
    """Partition layout r -> original hd dim. Pairs are (r, r+16) within
    each 32-block: block b holds pairs 16b..16b+15; r%32<16 -> real (even
    dim), else imag (odd dim)."""
    perm = np.empty(HD, dtype=np.int64)
    for r in range(HD):
        blk, j = divmod(r, 32)
        pair = blk * 16 + (j % 16)
        imag = j // 16
        perm[r] = 2 * pair + imag
    return perm


_PERM64 = _rope_perm64()


def prep_core_inputs(x, w_qkv, b_qkv, w_out, core, xt_cache):
    b = core // 4
    heads = [(core % 4) * HPC + i for i in range(HPC)]

    if b not in xt_cache:
        xb = np.asarray(x[b])  # [S, D]
        xt = np.ascontiguousarray(
            xb.T.reshape(KT, P, S).transpose(1, 0, 2).reshape(P, KT * S)
        )
        xt_cache[b] = _bf16(xt)
    xt = xt_cache[b]

    # permuted row indices for Q/K packs: pack pk rows = heads 2pk, 2pk+1
    def pack_rows(section, pk):
        rows = []
        for half in range(2):
            h = heads[2 * pk + half]
            rows.extend(section * D + h * HD + _PERM64)
        return rows

    def wT_tiles(rows):
        w_sel = w_qkv[rows]  # [128, 1024]
        # lhsT[p, k*128 + c] = w_sel[c, k*128+p]
        return np.ascontiguousarray(
            w_sel.T.reshape(KT, P, P).transpose(1, 0, 2).reshape(P, KT * P)
        )

    wq = np.concatenate([wT_tiles(pack_rows(0, pk)) for pk in range(NPAIR)], axis=1)
    wk = np.concatenate([wT_tiles(pack_rows(1, pk)) for pk in range(NPAIR)], axis=1)

    # V natural: rhs tiles [128 (D-chunk), 256 (4 heads x 64 natural)]
    vrows = []
    for h in heads:
        vrows.extend(range(2 * D + h * HD, 2 * D + (h + 1) * HD))
    wv_sel = w_qkv[vrows]  # [256, 1024]
    wv = np.ascontiguousarray(
        wv_sel.T.reshape(KT, P, 256).transpose(1, 0, 2).reshape(P, KT * 256)
    )

    # bias columns [128, 4]: (Q pk0, Q pk1, K pk0, K pk1)
    bqk = np.empty((P, 4), dtype=np.float32)
    for qk in range(2):
        for pk in range(NPAIR):
            rows = pack_rows(qk, pk)
            bqk[:, qk * 2 + pk] = b_qkv[rows]

    # rope tables in permuted-partition layout [128, S]
    dims = np.arange(0, HD, 2, dtype=np.float64)
    invf = 1.0 / (THETA ** (dims / HD))  # [32] per pair index
    pos = np.arange(S, dtype=np.float64)
    r = np.arange(HD)
    blk, j = r // 32, r % 32
    pair = blk * 16 + (j % 16)
    is_imag = (j % 32) >= 16
    freq = invf[pair]  # [64]
    ang = pos[None, :] * freq[:, None]  # [64, S]
    c64 = np.cos(ang)
    # S[r] = -sin for real, +sin for imag; S''[r] = S[partner(r)] = flipped
    s64 = np.sin(ang) * np.where(is_imag, -1.0, 1.0)[:, None]
    ropec = np.tile(c64, (2, 1))  # [128, S]
    ropes = np.tile(s64, (2, 1))

    trimask = np.tile(np.triu(np.ones((P, P), dtype=np.float32)), (1, 2))

    # wo[kk, p2*D + n] = w_out[n, gh*64 + kk%64], gh = heads[2*p2 + kk//64]
    wo = np.empty((P, NPAIR * D), dtype=np.float32)
    for p2 in range(NPAIR):
        for half in range(2):
            gh = heads[2 * p2 + half]
            wo[half * 64 : (half + 1) * 64, p2 * D : (p2 + 1) * D] = w_out[
                :, gh * HD : (gh + 1) * HD
            ].T
    return {
        "xt": xt,
        "wq": _bf16(wq),
        "wk": _bf16(wk),
        "wv": _bf16(wv),
        "bqk": np.ascontiguousarray(bqk),
        "ropec": _bf16(ropec),
        "ropes": _bf16(ropes),
        "trimask": _bf16(trimask),
        "wo": _bf16(wo),
    }


def kernel(x, w_qkv, b_qkv, w_out, b_out):
    global LAST_RESULTS
    x = np.asarray(x, dtype=np.float32)
    w_qkv = np.asarray(w_qkv, dtype=np.float32)
    b_qkv = np.asarray(b_qkv, dtype=np.float32)
    w_out = np.asarray(w_out, dtype=np.float32)
    b_out = np.asarray(b_out, dtype=np.float32)

    nc = get_program()
    xt_cache = {}
    in_maps = [
        prep_core_inputs(x, w_qkv, b_qkv, w_out, core, xt_cache)
        for core in range(NCORES)
    ]
    res = bass_utils.run_bass_kernel_spmd(
        nc, in_maps, core_ids=list(range(NCORES)), trace=TRACE
    )
    LAST_RESULTS = res
    partials = [r["outp"] for r in res.results]
    # v-bias contribution is constant across s (sum_k attn = 1):
    bconst = b_out + b_qkv[2 * D : 3 * D] @ w_out.T
    out = np.stack(
        [
            partials[0] + partials[1] + partials[2] + partials[3],
            partials[4] + partials[5] + partials[6] + partials[7],
        ]
    )
    out = out + bconst[None, None, :]
    return out.astype(np.float32)


# revision 21
# speedup vs baseline: 1.0062x; 1.0062x over previous
# Multi-head attention (RoPE, causal) Trainium2 Bass kernel, v2.
# B=2, S=2048, D=1024, 16 heads, hd=64, fp32 I/O.
#
# Sharding: 32 (batch, head) units over 8 cores -> each core gets one batch
# and 4 heads. Each core computes its 4 heads' attention output and the
# partial out-projection (sum over its heads); the host sums the 4 partials
# per batch and adds the bias constant.
#
# v2 vs v1: Q^T/K^T are produced directly transposed by the QKV projection
# (W stationary, X^T streaming) so no PE transposes are needed; RoPE runs in
# the transposed [hd, s] layout using a host-side W-row permutation that
# makes the rotation partner swap a single DVE stream_shuffle (swap the
# 16-halves of each 32-partition block); score matmuls are K=64 row-packed
# pairs (partitions 0:64 / 64:128) that execute concurrently on the PE;
# diagonal score blocks are N-trimmed; A/B phases are interleaved per
# 512-chunk to keep the PE dense.
#
# Self-contained: all shapes/sharding hardcoded; no sibling imports.

import numpy as np

import concourse.bass as bass  # noqa: F401
import concourse.mybir as mybir
import concourse.tile as tile
from concourse import bacc, bass_utils

F32 = mybir.dt.float32
BF16 = mybir.dt.bfloat16
EXP = mybir.ActivationFunctionType.Exp
ADD = mybir.AluOpType.add
MULT = mybir.AluOpType.mult

B = 2
S = 2048
D = 1024
NHEADS = 16
HD = 64
HPC = 4  # heads per core
NCORES = 8
NPAIR = 2  # head pairs per core
P = 128
CH = 512  # q chunk
NCH = S // CH  # 4
NT = S // P  # 16
KT = D // P  # 8
THETA = 10000.0

# swap the 16-halves of each 32-partition block (RoPE partner swap)
SWAP_MASK = list(range(16, 32)) + list(range(0, 16))

# module-level knobs for test harness
TRACE = False
LAST_RESULTS = None

_PROGRAM_CACHE = {}


def build_program():
    nc = bacc.Bacc(
        "TRN2", target_bir_lowering=False, debug=False, enable_asserts=False
    )

    # ---- DRAM I/O ----
    xt_d = nc.dram_tensor("xt", [P, KT * S], BF16, kind="ExternalInput").ap()
    wq_d = nc.dram_tensor("wq", [P, NPAIR * KT * P], BF16, kind="ExternalInput").ap()
    wk_d = nc.dram_tensor("wk", [P, NPAIR * KT * P], BF16, kind="ExternalInput").ap()
    wv_d = nc.dram_tensor("wv", [P, KT * 256], BF16, kind="ExternalInput").ap()
    bqk_d = nc.dram_tensor("bqk", [P, 4], F32, kind="ExternalInput").ap()
    ropec_d = nc.dram_tensor("ropec", [P, S], BF16, kind="ExternalInput").ap()
    ropes_d = nc.dram_tensor("ropes", [P, S], BF16, kind="ExternalInput").ap()
    trimask_d = nc.dram_tensor("trimask", [P, 2 * P], BF16, kind="ExternalInput").ap()
    wo_d = nc.dram_tensor("wo", [P, NPAIR * D], BF16, kind="ExternalInput").ap()
    out_d = nc.dram_tensor("outp", [S, D], F32, kind="ExternalOutput").ap()

    from contextlib import ExitStack

    with tile.TileContext(nc) as tc, ExitStack() as ctx:
        const = ctx.enter_context(tc.tile_pool(name="const", bufs=1))

        # weights for Q^T/K^T: lhsT tiles [128 (D-chunk k), 128 (pack rows)]
        wq_sb = const.tile([P, NPAIR * KT * P], BF16)
        wk_sb = const.tile([P, NPAIR * KT * P], BF16)
        wv_sb = const.tile([P, KT * 256], BF16)
        bqk_sb = const.tile([P, 4], F32)
        ropec_sb = const.tile([P, S], BF16)
        ropes_sb = const.tile([P, S], BF16)
        trimask_sb = const.tile([P, 2 * P], BF16)
        wo_sb = const.tile([P, NPAIR * D], BF16)
        # x^T, entire input: [128 (D-chunk k), k*S + t]
        xt_sb = const.tile([P, KT * S], BF16)

        # const loads: split across the two HWDGE rings; first-needed first.
        # sync ring: wq pack0, x chunk 0 k-slices, wq pack1, rest of x
        nc.sync.dma_start(wq_sb[:, 0 : KT * P], wq_d[:, 0 : KT * P])
        for k in range(KT):
            sl = slice(k * S, k * S + CH)
            nc.sync.dma_start(xt_sb[:, sl], xt_d[:, sl])
        nc.sync.dma_start(wq_sb[:, KT * P :], wq_d[:, KT * P :])
        for c in range(1, NCH):
            for k in range(KT):
                sl = slice(k * S + c * CH, k * S + (c + 1) * CH)
                nc.sync.dma_start(xt_sb[:, sl], xt_d[:, sl])
        # scalar/ACT ring: everything else
        nc.scalar.dma_start(wk_sb[:], wk_d[:])
        nc.scalar.dma_start(bqk_sb[:], bqk_d[:])
        nc.scalar.dma_start(ropec_sb[:], ropec_d[:])
        nc.scalar.dma_start(ropes_sb[:], ropes_d[:])
        nc.scalar.dma_start(wv_sb[:], wv_d[:])
        nc.scalar.dma_start(trimask_sb[:], trimask_d[:])
        nc.scalar.dma_start(wo_sb[:], wo_d[:])

        # persistent activations
        # Q^T / K^T packs: [128 (2 heads x 64 permuted hd), s] per pack
        qt = const.tile([P, NPAIR * S], BF16)
        kt_sb = const.tile([P, NPAIR * S], BF16)
        # vone: per s-tile [128, 4*65]; per head 64 V cols + ones col
        vone = const.tile([P, NT * (HPC * 65)], BF16)
        # O^T packs (normalized in place): [128 (2 heads*hd), s] per pair
        otn2 = const.tile([P, NPAIR * S], BF16)

        vone_v = vone.rearrange("p (t h c) -> p t h c", t=NT, h=HPC)
        nc.gpsimd.memset(vone_v[:, :, :, 64], 1.0)

        # half-selector columns for the rank-1 denominator broadcast
        halfsel = const.tile([1, 256], BF16)
        nc.gpsimd.memset(halfsel[:, 0:64], 1.0)
        nc.gpsimd.memset(halfsel[:, 64:192], 0.0)
        nc.gpsimd.memset(halfsel[:, 192:256], 1.0)

        # softmax denominator collection: per (pair, J): [headA 512 | headB 512]
        dall = const.tile([1, NPAIR * NCH * 1024], F32)
        dallinv = const.tile([1, NPAIR * NCH * 1024], BF16)

        bpsum = ctx.enter_context(tc.tile_pool(name="bpsum", bufs=2, space="PSUM"))
        opsum = ctx.enter_context(tc.tile_pool(name="opsum", bufs=1, space="PSUM"))
        awork = ctx.enter_context(tc.tile_pool(name="awork", bufs=3))
        bwork = ctx.enter_context(tc.tile_pool(name="bwork", bufs=4))
        fxwork = ctx.enter_context(tc.tile_pool(name="fxwork", bufs=2))
        cwork = ctx.enter_context(tc.tile_pool(name="cwork", bufs=3))

        # ---------------- Phase A ----------------
        def emit_qkT(c, pk, w_sb, bias_col, dst):
            """One 512-col chunk of a Q^T or K^T pack, with bias + RoPE."""
            ps = bpsum.tile([P, CH], F32, name="psq", tag="proj")
            for k in range(KT):
                nc.tensor.matmul(
                    ps[:],
                    lhsT=w_sb[:, (pk * KT + k) * P : (pk * KT + k + 1) * P],
                    rhs=xt_sb[:, k * S + c * CH : k * S + (c + 1) * CH],
                    start=(k == 0),
                    stop=(k == KT - 1),
                )
            # stage = ps + bias (per-partition)
            stage = awork.tile([P, CH], BF16, name="stage")
            nc.vector.tensor_scalar_add(stage[:], ps[:], bqk_sb[:, bias_col : bias_col + 1])
            # t1 = stage * S'' ; sw = shuffle(t1) ; rot = stage*C + sw
            t1 = awork.tile([P, CH], BF16, name="t1")
            nc.vector.tensor_mul(t1[:], stage[:], ropes_sb[:, c * CH : (c + 1) * CH])
            sw = awork.tile([P, CH], BF16, name="sw")
            nc.vector.stream_shuffle(sw[:], t1[:], SWAP_MASK)
            rot1 = awork.tile([P, CH], BF16, name="rot1")
            nc.vector.tensor_mul(rot1[:], stage[:], ropec_sb[:, c * CH : (c + 1) * CH])
            nc.vector.tensor_add(
                dst[:, pk * S + c * CH : pk * S + (c + 1) * CH], rot1[:], sw[:]
            )

        def emit_V(st):
            psv = bpsum.tile([P, 256], F32, name="psv", tag="proj")
            for k in range(KT):
                nc.tensor.matmul(
                    psv[:],
                    lhsT=xt_sb[:, k * S + st * P : k * S + (st + 1) * P],
                    rhs=wv_sb[:, k * 256 : (k + 1) * 256],
                    start=(k == 0),
                    stop=(k == KT - 1),
                )
            nc.scalar.copy(
                vone_v[:, st, :, 0:64],
                psv.rearrange("p (h c) -> p h c", h=HPC),
            )

        # ---------------- Phase B: attention ----------------
        def emit_BJ(p, j, a_units=()):
            """One (pair, q-chunk) attention pass. a_units: deferred Phase-A
            sub-unit closures sprinkled between m-iterations so the PE's
            A-work fills the slack under ACT-bound exp."""
            units = list(a_units)
            n_done = 0
            q_pack = qt[:, p * S : (p + 1) * S]
            k_pack = kt_sb[:, p * S : (p + 1) * S]
            ot2 = opsum.tile([P, 1024], F32, name="ot2", tag="acc")
            mlast = 4 * j + 3
            n_m = 4 * j + 4
            for m in range(n_m):
                off = m * P - j * CH if m >= 4 * j else 0
                sc2 = bpsum.tile([P, 1024], F32, name="sc2", tag="sc")
                nc.tensor.matmul(
                    sc2[:, off:CH],
                    lhsT=k_pack[0:64, m * P : (m + 1) * P],
                    rhs=q_pack[0:64, j * CH + off : (j + 1) * CH],
                )
                nc.tensor.matmul(
                    sc2[:, CH + off : 1024],
                    lhsT=k_pack[64:128, m * P : (m + 1) * P],
                    rhs=q_pack[64:128, j * CH + off : (j + 1) * CH],
                )
                at2 = bwork.tile([P, 1024], BF16, name="at2")
                if off > 0:
                    sc_v = sc2.rearrange("p (h q) -> p h q", h=2)
                    at_v = at2.rearrange("p (h q) -> p h q", h=2)
                    nc.scalar.activation(
                        at_v[:, :, off:CH], sc_v[:, :, off:CH], EXP, scale=0.125
                    )
                else:
                    nc.scalar.activation(at2[:], sc2[:], EXP, scale=0.125)
                if m >= 4 * j:
                    at_m = at2.rearrange("p (h q) -> p h q", h=2)[
                        :, :, off : off + P
                    ]
                    nc.vector.tensor_mul(
                        at_m,
                        at_m,
                        trimask_sb.rearrange("p (h q) -> p h q", h=2),
                    )
                nc.tensor.matmul(
                    ot2[0:65, off:CH],
                    lhsT=vone_v[:, m, 2 * p, :],
                    rhs=at2[:, off:CH],
                    start=(m == 0),
                    stop=(m == mlast),
                )
                nc.tensor.matmul(
                    ot2[0:65, CH + off : 1024],
                    lhsT=vone_v[:, m, 2 * p + 1, :],
                    rhs=at2[:, CH + off : 1024],
                    start=(m == 0),
                    stop=(m == mlast),
                )
                while n_done < len(units) * (m + 1) // n_m:
                    units[n_done]()
                    n_done += 1
            # ---- evict OT halves + denominators (per J) ----
            ctx_hp = tc.high_priority(offset=600)
            ctx_hp.__enter__()
            nc.scalar.copy(
                otn2[0:64, p * S + j * CH : p * S + (j + 1) * CH], ot2[0:64, 0:CH]
            )
            stgB = fxwork.tile([64, CH], BF16, name="stgB")
            nc.vector.tensor_copy(stgB[:], ot2[0:64, CH:1024])
            nc.sync.dma_start(
                otn2[64:128, p * S + j * CH : p * S + (j + 1) * CH], stgB[:]
            )
            dslot = (p * NCH + j) * 1024
            nc.vector.tensor_copy(
                dall[0:1, dslot : dslot + CH], ot2[64:65, 0:CH]
            )
            nc.scalar.copy(
                dall[0:1, dslot + CH : dslot + 1024], ot2[64:65, CH:1024]
            )
            # reciprocal on the denominator row in place: one custom-DVE op
            # (~18-bit accurate; denominators are >= 1 so edge cases are
            # safe), then downcast -- no DMA roundtrip.
            dvi32 = fxwork.tile([1, 1024], F32, name="dvi32")
            nc.vector.reciprocal_approx_fast(
                out=dvi32[0:1, :], in_=dall[0:1, dslot : dslot + 1024]
            )
            with nc.allow_low_precision("softmax denominators"):
                nc.vector.tensor_copy(
                    dallinv[0:1, dslot : dslot + 1024], dvi32[0:1, :]
                )
            ctx_hp.__exit__(None, None, None)

        def emit_final(p, j):
            dslot = (p * NCH + j) * 1024
            dvb = bpsum.tile([P, CH], F32, name="dvb", tag="proj")
            nc.tensor.matmul(
                dvb[:],
                lhsT=halfsel[0:1, 0:128],
                rhs=dallinv[0:1, dslot : dslot + CH],
                start=True,
                stop=False,
            )
            nc.tensor.matmul(
                dvb[:],
                lhsT=halfsel[0:1, 128:256],
                rhs=dallinv[0:1, dslot + CH : dslot + 1024],
                start=False,
                stop=True,
            )
            nc.vector.tensor_mul(
                otn2[:, p * S + j * CH : p * S + (j + 1) * CH],
                otn2[:, p * S + j * CH : p * S + (j + 1) * CH],
                dvb[:],
            )

        def emit_C(g):
            # out projection for q-tiles 4g..4g+3
            for qt_i in range(4 * g, 4 * g + 4):
                outsb = cwork.tile([P, D], F32, name="outsb")
                for dc in range(2):
                    pr = bpsum.tile([P, CH], F32, name="pr", tag="proj")
                    for p in range(NPAIR):
                        nc.tensor.matmul(
                            pr[:],
                            lhsT=otn2[:, p * S + qt_i * P : p * S + (qt_i + 1) * P],
                            rhs=wo_sb[:, p * D + dc * CH : p * D + (dc + 1) * CH],
                            start=(p == 0),
                            stop=(p == NPAIR - 1),
                        )
                    if dc == 0:
                        nc.vector.tensor_copy(outsb[:, 0:CH], pr[:])
                    else:
                        nc.scalar.copy(outsb[:, CH:D], pr[:])
                nc.gpsimd.dma_start(out_d[qt_i * P : (qt_i + 1) * P, :], outsb[:])

        # ---------------- schedule ----------------
        # A(0) pack0+V prefix; the rest of Phase A is deferred as sub-unit
        # closures sprinkled into the B m-loops (PE fills the slack under
        # ACT-bound exp). Chunk c's pack0+V units finish inside B(*, c-1)
        # before B(0, c); pack1 units before B(1, c). Finals pipeline one
        # (p, j) behind; out-projection per chunk after pair-1 finals.
        def unit_qkT(c, pk, w_sb, bias_col, dst):
            return lambda: emit_qkT(c, pk, w_sb, bias_col, dst)

        def unit_V(st):
            return lambda: emit_V(st)

        def units_p0(c):
            return [
                unit_qkT(c, 0, wq_sb, 0, qt),
                unit_qkT(c, 0, wk_sb, 2, kt_sb),
            ] + [unit_V(st) for st in range(4 * c, 4 * c + 4)]

        def units_p1(c):
            return [
                unit_qkT(c, 1, wq_sb, 1, qt),
                unit_qkT(c, 1, wk_sb, 3, kt_sb),
            ]

        emit_qkT(0, 0, wq_sb, 0, qt)
        emit_qkT(0, 0, wk_sb, 2, kt_sb)
        for st in range(4):
            emit_V(st)
        pending = None

        def after_B(p, j):
            nonlocal pending
            if pending is not None:
                emit_final(*pending)
                if pending[0] == 1:
                    emit_C(pending[1])
            pending = (p, j)

        emit_qkT(0, 1, wq_sb, 1, qt)
        emit_qkT(0, 1, wk_sb, 3, kt_sb)
        for j in range(NCH - 1):
            emit_BJ(0, j)
            after_B(0, j)
            for u in units_p0(j + 1):
                u()
            emit_BJ(1, j)
            after_B(1, j)
            for u in units_p1(j + 1):
                u()
        # All out-projection chunks are deferred into the j=3 region: their
        # matmuls are always-ready fillers for the PE stalls of the pure-B
        # last chunk (no Phase-A work remains there) and cover the last
        # fixup chains' latency, keeping the PE warm into C(3).
        emit_BJ(0, NCH - 1)
        after_B(0, NCH - 1)
        emit_BJ(1, NCH - 1)
        after_B(1, NCH - 1)
        emit_final(*pending)
        emit_C(pending[1])

    nc.compile()
    return nc


def get_program():
    if "v2" not in _PROGRAM_CACHE:
        _PROGRAM_CACHE["v2"] = build_program()
    return _PROGRAM_CACHE["v2"]


def _bf16(a):
    import ml_dtypes

    return np.ascontiguousarray(a).astype(ml_dtypes.bfloat16)


def _rope_perm64():
    """Partition layout r -> original hd dim. Pairs are (r, r+16) within
    each 32-block: block b holds pairs 16b..16b+15; r%32<16 -> real (even
    dim), else imag (odd dim)."""
    perm = np.empty(HD, dtype=np.int64)
    for r in range(HD):
        blk, j = divmod(r, 32)
        pair = blk * 16 + (j % 16)
        imag = j // 16
        perm[r] = 2 * pair + imag
    return perm


_PERM64 = _rope_perm64()


def prep_core_inputs(x, w_qkv, b_qkv, w_out, core, xt_cache):
    b = core // 4
    heads = [(core % 4) * HPC + i for i in range(HPC)]

    if b not in xt_cache:
        xb = np.asarray(x[b])  # [S, D]
        xt = np.ascontiguousarray(
            xb.T.reshape(KT, P, S).transpose(1, 0, 2).reshape(P, KT * S)
        )
        xt_cache[b] = _bf16(xt)
    xt = xt_cache[b]

    # permuted row indices for Q/K packs: pack pk rows = heads 2pk, 2pk+1
    def pack_rows(section, pk):
        rows = []
        for half in range(2):
            h = heads[2 * pk + half]
            rows.extend(section * D + h * HD + _PERM64)
        return rows

    def wT_tiles(rows):
        w_sel = w_qkv[rows]  # [128, 1024]
        # lhsT[p, k*128 + c] = w_sel[c, k*128+p]
        return np.ascontiguousarray(
            w_sel.T.reshape(KT, P, P).transpose(1, 0, 2).reshape(P, KT * P)
        )

    wq = np.concatenate([wT_tiles(pack_rows(0, pk)) for pk in range(NPAIR)], axis=1)
    wk = np.concatenate([wT_tiles(pack_rows(1, pk)) for pk in range(NPAIR)], axis=1)

    # V natural: rhs tiles [128 (D-chunk), 256 (4 heads x 64 natural)]
    vrows = []
    for h in heads:
        vrows.extend(range(2 * D + h * HD, 2 * D + (h + 1) * HD))
    wv_sel = w_qkv[vrows]  # [256, 1024]
    wv = np.ascontiguousarray(
        wv_sel.T.reshape(KT, P, 256).transpose(1, 0, 2).reshape(P, KT * 256)
    )

    # bias columns [128, 4]: (Q pk0, Q pk1, K pk0, K pk1)
    bqk = np.empty((P, 4), dtype=np.float32)
    for qk in range(2):
        for pk in range(NPAIR):
            rows = pack_rows(qk, pk)
            bqk[:, qk * 2 + pk] = b_qkv[rows]

    # rope tables in permuted-partition layout [128, S]
    dims = np.arange(0, HD, 2, dtype=np.float64)
    invf = 1.0 / (THETA ** (dims / HD))  # [32] per pair index
    pos = np.arange(S, dtype=np.float64)
    r = np.arange(HD)
    blk, j = r // 32, r % 32
    pair = blk * 16 + (j % 16)
    is_imag = (j % 32) >= 16
    freq = invf[pair]  # [64]
    ang = pos[None, :] * freq[:, None]  # [64, S]
    c64 = np.cos(ang)
    # S[r] = -sin for real, +sin for imag; S''[r] = S[partner(r)] = flipped
    s64 = np.sin(ang) * np.where(is_imag, -1.0, 1.0)[:, None]
    ropec = np.tile(c64, (2, 1))  # [128, S]
    ropes = np.tile(s64, (2, 1))

    trimask = np.tile(np.triu(np.ones((P, P), dtype=np.float32)), (1, 2))

    # wo[kk, p2*D + n] = w_out[n, gh*64 + kk%64], gh = heads[2*p2 + kk//64]
    wo = np.empty((P, NPAIR * D), dtype=np.float32)
    for p2 in range(NPAIR):
        for half in range(2):
            gh = heads[2 * p2 + half]
            wo[half * 64 : (half + 1) * 64, p2 * D : (p2 + 1) * D] = w_out[
                :, gh * HD : (gh + 1) * HD
            ].T
    return {
        "xt": xt,
        "wq": _bf16(wq),
        "wk": _bf16(wk),
        "wv": _bf16(wv),
        "bqk": np.ascontiguousarray(bqk),
        "ropec": _bf16(ropec),
        "ropes": _bf16(ropes),
        "trimask": _bf16(trimask),
        "wo": _bf16(wo),
    }


def kernel(x, w_qkv, b_qkv, w_out, b_out):
    global LAST_RESULTS
    x = np.asarray(x, dtype=np.float32)
    w_qkv = np.asarray(w_qkv, dtype=np.float32)
    b_qkv = np.asarray(b_qkv, dtype=np.float32)
    w_out = np.asarray(w_out, dtype=np.float32)
    b_out = np.asarray(b_out, dtype=np.float32)

    nc = get_program()
    xt_cache = {}
    in_maps = [
        prep_core_inputs(x, w_qkv, b_qkv, w_out, core, xt_cache)
        for core in range(NCORES)
    ]
    res = bass_utils.run_bass_kernel_spmd(
        nc, in_maps, core_ids=list(range(NCORES)), trace=TRACE
    )
    LAST_RESULTS = res
    partials = [r["outp"] for r in res.results]
    # v-bias contribution is constant across s (sum_k attn = 1):
    bconst = b_out + b_qkv[2 * D : 3 * D] @ w_out.T
    out = np.stack(
        [
            partials[0] + partials[1] + partials[2] + partials[3],
            partials[4] + partials[5] + partials[6] + partials[7],
        ]
    )
    out = out + bconst[None, None, :]
    return out.astype(np.float32)


# revision 22
# speedup vs baseline: 1.0156x; 1.0093x over previous
# Multi-head attention (RoPE, causal) Trainium2 Bass kernel, v2.
# B=2, S=2048, D=1024, 16 heads, hd=64, fp32 I/O.
#
# Sharding: 32 (batch, head) units over 8 cores -> each core gets one batch
# and 4 heads. Each core computes its 4 heads' attention output and the
# partial out-projection (sum over its heads); the host sums the 4 partials
# per batch and adds the bias constant.
#
# v2 vs v1: Q^T/K^T are produced directly transposed by the QKV projection
# (W stationary, X^T streaming) so no PE transposes are needed; RoPE runs in
# the transposed [hd, s] layout using a host-side W-row permutation that
# makes the rotation partner swap a single DVE stream_shuffle (swap the
# 16-halves of each 32-partition block); score matmuls are K=64 row-packed
# pairs (partitions 0:64 / 64:128) that execute concurrently on the PE;
# diagonal score blocks are N-trimmed; A/B phases are interleaved per
# 512-chunk to keep the PE dense.
#
# Self-contained: all shapes/sharding hardcoded; no sibling imports.

import numpy as np

import concourse.bass as bass  # noqa: F401
import concourse.mybir as mybir
import concourse.tile as tile
from concourse import bacc, bass_utils

F32 = mybir.dt.float32
BF16 = mybir.dt.bfloat16
EXP = mybir.ActivationFunctionType.Exp
ADD = mybir.AluOpType.add
MULT = mybir.AluOpType.mult

B = 2
S = 2048
D = 1024
NHEADS = 16
HD = 64
HPC = 4  # heads per core
NCORES = 8
NPAIR = 2  # head pairs per core
P = 128
CH = 512  # q chunk
NCH = S // CH  # 4
NT = S // P  # 16
KT = D // P  # 8
THETA = 10000.0

# swap the 16-halves of each 32-partition block (RoPE partner swap)
SWAP_MASK = list(range(16, 32)) + list(range(0, 16))

# module-level knobs for test harness
TRACE = False
LAST_RESULTS = None

_PROGRAM_CACHE = {}


def build_program():
    nc = bacc.Bacc(
        "TRN2", target_bir_lowering=False, debug=False, enable_asserts=False
    )

    # ---- DRAM I/O ----
    xt_d = nc.dram_tensor("xt", [P, KT * S], BF16, kind="ExternalInput").ap()
    wq_d = nc.dram_tensor("wq", [P, NPAIR * KT * P], BF16, kind="ExternalInput").ap()
    wk_d = nc.dram_tensor("wk", [P, NPAIR * KT * P], BF16, kind="ExternalInput").ap()
    wv_d = nc.dram_tensor("wv", [P, KT * 256], BF16, kind="ExternalInput").ap()
    bqk_d = nc.dram_tensor("bqk", [P, 4], F32, kind="ExternalInput").ap()
    ropec_d = nc.dram_tensor("ropec", [P, S], BF16, kind="ExternalInput").ap()
    ropes_d = nc.dram_tensor("ropes", [P, S], BF16, kind="ExternalInput").ap()
    trimask_d = nc.dram_tensor("trimask", [P, 2 * P], BF16, kind="ExternalInput").ap()
    wo_d = nc.dram_tensor("wo", [P, NPAIR * D], BF16, kind="ExternalInput").ap()
    out_d = nc.dram_tensor("outp", [S, D], F32, kind="ExternalOutput").ap()

    from contextlib import ExitStack

    with tile.TileContext(nc) as tc, ExitStack() as ctx:
        const = ctx.enter_context(tc.tile_pool(name="const", bufs=1))

        # weights for Q^T/K^T: lhsT tiles [128 (D-chunk k), 128 (pack rows)]
        wq_sb = const.tile([P, NPAIR * KT * P], BF16)
        wk_sb = const.tile([P, NPAIR * KT * P], BF16)
        wv_sb = const.tile([P, KT * 256], BF16)
        bqk_sb = const.tile([P, 4], F32)
        ropec_sb = const.tile([P, S], BF16)
        ropes_sb = const.tile([P, S], BF16)
        trimask_sb = const.tile([P, 2 * P], BF16)
        wo_sb = const.tile([P, NPAIR * D], BF16)
        # x^T, entire input: [128 (D-chunk k), k*S + t]
        xt_sb = const.tile([P, KT * S], BF16)

        # const loads: split across the two HWDGE rings; first-needed first.
        # sync ring: wq pack0, x chunk 0 k-slices, wq pack1, rest of x
        nc.sync.dma_start(wq_sb[:, 0 : KT * P], wq_d[:, 0 : KT * P])
        for k in range(KT):
            sl = slice(k * S, k * S + CH)
            nc.sync.dma_start(xt_sb[:, sl], xt_d[:, sl])
        nc.sync.dma_start(wq_sb[:, KT * P :], wq_d[:, KT * P :])
        for c in range(1, NCH):
            for k in range(KT):
                sl = slice(k * S + c * CH, k * S + (c + 1) * CH)
                nc.sync.dma_start(xt_sb[:, sl], xt_d[:, sl])
        # scalar/ACT ring: everything else
        nc.scalar.dma_start(wk_sb[:], wk_d[:])
        nc.scalar.dma_start(bqk_sb[:], bqk_d[:])
        nc.scalar.dma_start(ropec_sb[:], ropec_d[:])
        nc.scalar.dma_start(ropes_sb[:], ropes_d[:])
        nc.scalar.dma_start(wv_sb[:], wv_d[:])
        nc.scalar.dma_start(trimask_sb[:], trimask_d[:])
        nc.scalar.dma_start(wo_sb[:], wo_d[:])

        # persistent activations
        # Q^T / K^T packs: [128 (2 heads x 64 permuted hd), s] per pack
        qt = const.tile([P, NPAIR * S], BF16)
        kt_sb = const.tile([P, NPAIR * S], BF16)
        # vone: per s-tile [128, 4*65]; per head 64 V cols + ones col
        vone = const.tile([P, NT * (HPC * 65)], BF16)
        # O^T packs (normalized in place): [128 (2 heads*hd), s] per pair
        otn2 = const.tile([P, NPAIR * S], BF16)

        vone_v = vone.rearrange("p (t h c) -> p t h c", t=NT, h=HPC)
        nc.gpsimd.memset(vone_v[:, :, :, 64], 1.0)

        # half-selector columns for the rank-1 denominator broadcast
        halfsel = const.tile([1, 256], BF16)
        nc.gpsimd.memset(halfsel[:, 0:64], 1.0)
        nc.gpsimd.memset(halfsel[:, 64:192], 0.0)
        nc.gpsimd.memset(halfsel[:, 192:256], 1.0)

        # softmax denominator collection: per (pair, J): [headA 512 | headB 512]
        dall = const.tile([1, NPAIR * NCH * 1024], F32)
        dallinv = const.tile([1, NPAIR * NCH * 1024], BF16)

        bpsum = ctx.enter_context(tc.tile_pool(name="bpsum", bufs=2, space="PSUM"))
        opsum = ctx.enter_context(tc.tile_pool(name="opsum", bufs=1, space="PSUM"))
        awork = ctx.enter_context(tc.tile_pool(name="awork", bufs=3))
        bwork = ctx.enter_context(tc.tile_pool(name="bwork", bufs=4))
        fxwork = ctx.enter_context(tc.tile_pool(name="fxwork", bufs=2))
        cwork = ctx.enter_context(tc.tile_pool(name="cwork", bufs=3))

        # ---------------- Phase A ----------------
        def emit_qkT(c, pk, w_sb, bias_col, dst):
            """One 512-col chunk of a Q^T or K^T pack, with bias + RoPE."""
            ps = bpsum.tile([P, CH], F32, name="psq", tag="proj")
            for k in range(KT):
                nc.tensor.matmul(
                    ps[:],
                    lhsT=w_sb[:, (pk * KT + k) * P : (pk * KT + k + 1) * P],
                    rhs=xt_sb[:, k * S + c * CH : k * S + (c + 1) * CH],
                    start=(k == 0),
                    stop=(k == KT - 1),
                )
            # stage = ps + bias (per-partition)
            stage = awork.tile([P, CH], BF16, name="stage")
            nc.vector.tensor_scalar_add(stage[:], ps[:], bqk_sb[:, bias_col : bias_col + 1])
            # t1 = stage * S'' ; sw = shuffle(t1) ; rot = stage*C + sw
            t1 = awork.tile([P, CH], BF16, name="t1")
            nc.vector.tensor_mul(t1[:], stage[:], ropes_sb[:, c * CH : (c + 1) * CH])
            sw = awork.tile([P, CH], BF16, name="sw")
            nc.vector.stream_shuffle(sw[:], t1[:], SWAP_MASK)
            rot1 = awork.tile([P, CH], BF16, name="rot1")
            nc.vector.tensor_mul(rot1[:], stage[:], ropec_sb[:, c * CH : (c + 1) * CH])
            nc.vector.tensor_add(
                dst[:, pk * S + c * CH : pk * S + (c + 1) * CH], rot1[:], sw[:]
            )

        def emit_V(st):
            psv = bpsum.tile([P, 256], F32, name="psv", tag="proj")
            for k in range(KT):
                nc.tensor.matmul(
                    psv[:],
                    lhsT=xt_sb[:, k * S + st * P : k * S + (st + 1) * P],
                    rhs=wv_sb[:, k * 256 : (k + 1) * 256],
                    start=(k == 0),
                    stop=(k == KT - 1),
                )
            nc.scalar.copy(
                vone_v[:, st, :, 0:64],
                psv.rearrange("p (h c) -> p h c", h=HPC),
            )

        # ---------------- Phase B: attention ----------------
        def emit_BJ(p, j, a_units=()):
            """One (pair, q-chunk) attention pass. a_units: deferred Phase-A
            sub-unit closures sprinkled between m-iterations so the PE's
            A-work fills the slack under ACT-bound exp."""
            units = list(a_units)
            n_done = 0
            q_pack = qt[:, p * S : (p + 1) * S]
            k_pack = kt_sb[:, p * S : (p + 1) * S]
            ot2 = opsum.tile([P, 1024], F32, name="ot2", tag="acc")
            mlast = 4 * j + 3
            n_m = 4 * j + 4
            for m in range(n_m):
                off = m * P - j * CH if m >= 4 * j else 0
                sc2 = bpsum.tile([P, 1024], F32, name="sc2", tag="sc")
                nc.tensor.matmul(
                    sc2[:, off:CH],
                    lhsT=k_pack[0:64, m * P : (m + 1) * P],
                    rhs=q_pack[0:64, j * CH + off : (j + 1) * CH],
                )
                nc.tensor.matmul(
                    sc2[:, CH + off : 1024],
                    lhsT=k_pack[64:128, m * P : (m + 1) * P],
                    rhs=q_pack[64:128, j * CH + off : (j + 1) * CH],
                )
                at2 = bwork.tile([P, 1024], BF16, name="at2")
                if off > 0:
                    sc_v = sc2.rearrange("p (h q) -> p h q", h=2)
                    at_v = at2.rearrange("p (h q) -> p h q", h=2)
                    nc.scalar.activation(
                        at_v[:, :, off:CH], sc_v[:, :, off:CH], EXP, scale=0.125
                    )
                else:
                    nc.scalar.activation(at2[:], sc2[:], EXP, scale=0.125)
                if m >= 4 * j:
                    at_m = at2.rearrange("p (h q) -> p h q", h=2)[
                        :, :, off : off + P
                    ]
                    nc.vector.tensor_mul(
                        at_m,
                        at_m,
                        trimask_sb.rearrange("p (h q) -> p h q", h=2),
                    )
                nc.tensor.matmul(
                    ot2[0:65, off:CH],
                    lhsT=vone_v[:, m, 2 * p, :],
                    rhs=at2[:, off:CH],
                    start=(m == 0),
                    stop=(m == mlast),
                )
                nc.tensor.matmul(
                    ot2[0:65, CH + off : 1024],
                    lhsT=vone_v[:, m, 2 * p + 1, :],
                    rhs=at2[:, CH + off : 1024],
                    start=(m == 0),
                    stop=(m == mlast),
                )
                while n_done < len(units) * (m + 1) // n_m:
                    units[n_done]()
                    n_done += 1
            # ---- evict OT halves + denominators (per J) ----
            ctx_hp = tc.high_priority(offset=600)
            ctx_hp.__enter__()
            nc.scalar.copy(
                otn2[0:64, p * S + j * CH : p * S + (j + 1) * CH], ot2[0:64, 0:CH]
            )
            stgB = fxwork.tile([64, CH], BF16, name="stgB")
            nc.vector.tensor_copy(stgB[:], ot2[0:64, CH:1024])
            nc.sync.dma_start(
                otn2[64:128, p * S + j * CH : p * S + (j + 1) * CH], stgB[:]
            )
            dslot = (p * NCH + j) * 1024
            nc.vector.tensor_copy(
                dall[0:1, dslot : dslot + CH], ot2[64:65, 0:CH]
            )
            nc.scalar.copy(
                dall[0:1, dslot + CH : dslot + 1024], ot2[64:65, CH:1024]
            )
            # reciprocal on the denominator row in place: one custom-DVE op
            # (~18-bit accurate; denominators are >= 1 so edge cases are
            # safe), then downcast -- no DMA roundtrip.
            dvi32 = fxwork.tile([1, 1024], F32, name="dvi32")
            nc.vector.reciprocal_approx_fast(
                out=dvi32[0:1, :], in_=dall[0:1, dslot : dslot + 1024]
            )
            with nc.allow_low_precision("softmax denominators"):
                nc.vector.tensor_copy(
                    dallinv[0:1, dslot : dslot + 1024], dvi32[0:1, :]
                )
            ctx_hp.__exit__(None, None, None)

        def emit_final(p, j):
            dslot = (p * NCH + j) * 1024
            dvb = bpsum.tile([P, CH], F32, name="dvb", tag="proj")
            nc.tensor.matmul(
                dvb[:],
                lhsT=halfsel[0:1, 0:128],
                rhs=dallinv[0:1, dslot : dslot + CH],
                start=True,
                stop=False,
            )
            nc.tensor.matmul(
                dvb[:],
                lhsT=halfsel[0:1, 128:256],
                rhs=dallinv[0:1, dslot + CH : dslot + 1024],
                start=False,
                stop=True,
            )
            nc.vector.tensor_mul(
                otn2[:, p * S + j * CH : p * S + (j + 1) * CH],
                otn2[:, p * S + j * CH : p * S + (j + 1) * CH],
                dvb[:],
            )

        def emit_C(g):
            # out projection for q-tiles 4g..4g+3
            for qt_i in range(4 * g, 4 * g + 4):
                outsb = cwork.tile([P, D], F32, name="outsb")
                for dc in range(2):
                    pr = bpsum.tile([P, CH], F32, name="pr", tag="proj")
                    for p in range(NPAIR):
                        nc.tensor.matmul(
                            pr[:],
                            lhsT=otn2[:, p * S + qt_i * P : p * S + (qt_i + 1) * P],
                            rhs=wo_sb[:, p * D + dc * CH : p * D + (dc + 1) * CH],
                            start=(p == 0),
                            stop=(p == NPAIR - 1),
                        )
                    if dc == 0:
                        nc.vector.tensor_copy(outsb[:, 0:CH], pr[:])
                    else:
                        nc.scalar.copy(outsb[:, CH:D], pr[:])
                nc.sync.dma_start(out_d[qt_i * P : (qt_i + 1) * P, :], outsb[:])

        # ---------------- schedule ----------------
        # A(0) pack0+V prefix; the rest of Phase A is deferred as sub-unit
        # closures sprinkled into the B m-loops (PE fills the slack under
        # ACT-bound exp). Chunk c's pack0+V units finish inside B(*, c-1)
        # before B(0, c); pack1 units before B(1, c). Finals pipeline one
        # (p, j) behind; out-projection per chunk after pair-1 finals.
        def unit_qkT(c, pk, w_sb, bias_col, dst):
            return lambda: emit_qkT(c, pk, w_sb, bias_col, dst)

        def unit_V(st):
            return lambda: emit_V(st)

        def units_p0(c):
            return [
                unit_qkT(c, 0, wq_sb, 0, qt),
                unit_qkT(c, 0, wk_sb, 2, kt_sb),
            ] + [unit_V(st) for st in range(4 * c, 4 * c + 4)]

        def units_p1(c):
            return [
                unit_qkT(c, 1, wq_sb, 1, qt),
                unit_qkT(c, 1, wk_sb, 3, kt_sb),
            ]

        emit_qkT(0, 0, wq_sb, 0, qt)
        emit_qkT(0, 0, wk_sb, 2, kt_sb)
        for st in range(4):
            emit_V(st)
        pending = None

        def after_B(p, j):
            nonlocal pending
            if pending is not None:
                emit_final(*pending)
                if pending[0] == 1:
                    emit_C(pending[1])
            pending = (p, j)

        emit_qkT(0, 1, wq_sb, 1, qt)
        emit_qkT(0, 1, wk_sb, 3, kt_sb)
        for j in range(NCH - 1):
            emit_BJ(0, j)
            after_B(0, j)
            for u in units_p0(j + 1):
                u()
            emit_BJ(1, j)
            after_B(1, j)
            for u in units_p1(j + 1):
                u()
        # All out-projection chunks are deferred into the j=3 region: their
        # matmuls are always-ready fillers for the PE stalls of the pure-B
        # last chunk (no Phase-A work remains there) and cover the last
        # fixup chains' latency, keeping the PE warm into C(3).
        emit_BJ(0, NCH - 1)
        after_B(0, NCH - 1)
        emit_BJ(1, NCH - 1)
        after_B(1, NCH - 1)
        emit_final(*pending)
        emit_C(pending[1])

    nc.compile()
    return nc


def get_program():
    if "v2" not in _PROGRAM_CACHE:
        _PROGRAM_CACHE["v2"] = build_program()
    return _PROGRAM_CACHE["v2"]


def _bf16(a):
    import ml_dtypes

    return np.ascontiguousarray(a).astype(ml_dtypes.bfloat16)


def _rope_perm64():
    """Partition layout r -> original hd dim. Pairs are (r, r+16) within
    each 32-block: block b holds pairs 16b..16b+15; r%32<16 -> real (even
    dim), else imag (odd dim)."""
    perm = np.empty(HD, dtype=np.int64)
    for r in range(HD):
        blk, j = divmod(r, 32)
        pair = blk * 16 + (j % 16)
        imag = j // 16
        perm[r] = 2 * pair + imag
    return perm


_PERM64 = _rope_perm64()


def prep_core_inputs(x, w_qkv, b_qkv, w_out, core, xt_cache):
    b = core // 4
    heads = [(core % 4) * HPC + i for i in range(HPC)]

    if b not in xt_cache:
        xb = np.asarray(x[b])  # [S, D]
        xt = np.ascontiguousarray(
            xb.T.reshape(KT, P, S).transpose(1, 0, 2).reshape(P, KT * S)
        )
        xt_cache[b] = _bf16(xt)
    xt = xt_cache[b]

    # permuted row indices for Q/K packs: pack pk rows = heads 2pk, 2pk+1
    def pack_rows(section, pk):
        rows = []
        for half in range(2):
            h = heads[2 * pk + half]
            rows.extend(section * D + h * HD + _PERM64)
        return rows

    def wT_tiles(rows):
        w_sel = w_qkv[rows]  # [128, 1024]
        # lhsT[p, k*128 + c] = w_sel[c, k*128+p]
        return np.ascontiguousarray(
            w_sel.T.reshape(KT, P, P).transpose(1, 0, 2).reshape(P, KT * P)
        )

    wq = np.concatenate([wT_tiles(pack_rows(0, pk)) for pk in range(NPAIR)], axis=1)
    wk = np.concatenate([wT_tiles(pack_rows(1, pk)) for pk in range(NPAIR)], axis=1)

    # V natural: rhs tiles [128 (D-chunk), 256 (4 heads x 64 natural)]
    vrows = []
    for h in heads:
        vrows.extend(range(2 * D + h * HD, 2 * D + (h + 1) * HD))
    wv_sel = w_qkv[vrows]  # [256, 1024]
    wv = np.ascontiguousarray(
        wv_sel.T.reshape(KT, P, 256).transpose(1, 0, 2).reshape(P, KT * 256)
    )

    # bias columns [128, 4]: (Q pk0, Q pk1, K pk0, K pk1)
    bqk = np.empty((P, 4), dtype=np.float32)
    for qk in range(2):
        for pk in range(NPAIR):
            rows = pack_rows(qk, pk)
            bqk[:, qk * 2 + pk] = b_qkv[rows]

    # rope tables in permuted-partition layout [128, S]
    dims = np.arange(0, HD, 2, dtype=np.float64)
    invf = 1.0 / (THETA ** (dims / HD))  # [32] per pair index
    pos = np.arange(S, dtype=np.float64)
    r = np.arange(HD)
    blk, j = r // 32, r % 32
    pair = blk * 16 + (j % 16)
    is_imag = (j % 32) >= 16
    freq = invf[pair]  # [64]
    ang = pos[None, :] * freq[:, None]  # [64, S]
    c64 = np.cos(ang)
    # S[r] = -sin for real, +sin for imag; S''[r] = S[partner(r)] = flipped
    s64 = np.sin(ang) * np.where(is_imag, -1.0, 1.0)[:, None]
    ropec = np.tile(c64, (2, 1))  # [128, S]
    ropes = np.tile(s64, (2, 1))

    trimask = np.tile(np.triu(np.ones((P, P), dtype=np.float32)), (1, 2))

    # wo[kk, p2*D + n] = w_out[n, gh*64 + kk%64], gh = heads[2*p2 + kk//64]
    wo = np.empty((P, NPAIR * D), dtype=np.float32)
    for p2 in range(NPAIR):
        for half in range(2):
            gh = heads[2 * p2 + half]
            wo[half * 64 : (half + 1) * 64, p2 * D : (p2 + 1) * D] = w_out[
                :, gh * HD : (gh + 1) * HD
            ].T
    return {
        "xt": xt,
        "wq": _bf16(wq),
        "wk": _bf16(wk),
        "wv": _bf16(wv),
        "bqk": np.ascontiguousarray(bqk),
        "ropec": _bf16(ropec),
        "ropes": _bf16(ropes),
        "trimask": _bf16(trimask),
        "wo": _bf16(wo),
    }


def kernel(x, w_qkv, b_qkv, w_out, b_out):
    global LAST_RESULTS
    x = np.asarray(x, dtype=np.float32)
    w_qkv = np.asarray(w_qkv, dtype=np.float32)
    b_qkv = np.asarray(b_qkv, dtype=np.float32)
    w_out = np.asarray(w_out, dtype=np.float32)
    b_out = np.asarray(b_out, dtype=np.float32)

    nc = get_program()
    xt_cache = {}
    in_maps = [
        prep_core_inputs(x, w_qkv, b_qkv, w_out, core, xt_cache)
        for core in range(NCORES)
    ]
    res = bass_utils.run_bass_kernel_spmd(
        nc, in_maps, core_ids=list(range(NCORES)), trace=TRACE
    )
    LAST_RESULTS = res
    partials = [r["outp"] for r in res.results]
    # v-bias contribution is constant across s (sum_k attn = 1):
    bconst = b_out + b_qkv[2 * D : 3 * D] @ w_out.T
    out = np.stack(
        [
            partials[0] + partials[1] + partials[2] + partials[3],
            partials[4] + partials[5] + partials[6] + partials[7],
        ]
    )
    out = out + bconst[None, None, :]
    return out.astype(np.float32)
